# revision 9
# baseline (speedup 1.0000x reference)
"""Trainium2 Bass kernel for nn_Decoder_11278584119887 (self-contained).

6-layer dense transformer decoder with head-averaged attention weights.

Sharding: 8 NeuronCores = 4 batch elements x 2 sequence halves. Per layer,
chunked bf16 AllGathers (pair groups) publish the residual stream; every
core reads the full-L stream from the AllGather chunk buffers (global row
order, so the SPMD program is core-uniform).

Per-layer structure (per core, R=1024 own rows = 8 l-tiles):
  proj:   wq/wk resident; Q^T (own rows, from hout DRAM) and K^T (full L,
          from AG chunks) via DMA-transposed h; PSUM evacuated on ScalarE.
  sweep:  per l-tile: scores (PE, head pairs packed on 64-row groups) ->
          exp+rowsum (ACT) -> rescale+head-accumulate (DVE, tail heads on
          GpSimd) -> +rel, second exp (ACT) -> z^T (DMA xbar) -> sa (PE)
          -> res1+fused LN1/2 -> h2^T. FFN runs in 4 chunks of 2 l-tiles
          interleaved into the sweep so PE stays busy while ACT crunches
          softmax. res2+LN3 per tile; LN rstd = Exp(-0.5*Ln(v+eps)) keeps
          the whole kernel on one ACT table set.
  AG:     2 chunked AllGathers per layer, kicked as soon as their 4 l-tiles
          finish LN3, overlapping the FFN tail and next-layer Q/K proj.
"""

import sys as _sys

for _p in ("/root/.axon_site/_ro/trn_rl_repo", "/opt/trn_rl_repo"):
    if _p not in _sys.path:
        _sys.path.append(_p)

import math
from dataclasses import dataclass

import numpy as np

import concourse.bass as bass
import concourse.mybir as mybir
import concourse.tile as tile

F32 = mybir.dt.float32
BF16 = mybir.dt.bfloat16
AF = mybir.ActivationFunctionType
OP = mybir.AluOpType

P = 128
EPS = 1e-5
DECAY = 16.0
CSCALE = 1.0


@dataclass
class Cfg:
    B: int = 4
    L: int = 2048
    E: int = 1024
    H: int = 16
    DH: int = 64
    F: int = 4096
    D: int = 6
    n_pair: int = 2  # cores per batch element
    pool_heads: int = 3  # stage-1 heads accumulated on GpSimd

    @property
    def n_cores(self):
        return self.B * self.n_pair

    @property
    def R(self):
        return self.L // self.n_pair

    @property
    def LT(self):
        return self.R // P

    @property
    def MT(self):
        return self.L // P

    @property
    def ET(self):
        return self.E // P

    @property
    def FT(self):
        return self.F // P


FULL = Cfg()
TINY = Cfg(B=1, L=256, E=256, H=4, DH=64, F=512, D=2, n_pair=2, pool_heads=1)


def build_decoder(nc, cfg: Cfg, no_collective: bool = False):
    c = cfg
    ISD = 1.0 / math.sqrt(c.DH)
    CH = float(CSCALE / c.H)
    LT, MT, ET, FT = c.LT, c.MT, c.ET, c.FT
    L, R, E, F, H, D = c.L, c.R, c.E, c.F, c.H, c.D
    HPT = P // c.DH  # heads per e'-tile (2)
    NPH = min(c.pool_heads, max(0, H - 2))  # heads accumulated on Pool
    pool_set = {int((i + 1) * H / (NPH + 1)) for i in range(NPH)} if NPH else set()

    SC = min(512, R)            # proj moving-dim chunk (rows)
    NSC = L // SC               # proj chunks over full L
    NSCq = R // SC              # proj chunks over own rows
    SCCH = min(1024, L)         # scores psum chunk (2 banks)
    NCH = L // SCCH
    JW = min(512, SCCH)         # single-MM moving width
    NJ = SCCH // JW
    EH = min(512, E)            # sa e-chunk
    NEH = E // EH
    SAG = min(2, LT)            # l-tiles per sa group
    FTL = min(2, LT)            # l-tiles per FFN chunk
    NFC = LT // FTL             # FFN chunks
    Lh = FTL * P                # FFN moving width
    FH2 = min(16, FT)           # w2 block f-tiles
    NFH = FT // FH2
    AGC = 2 if LT >= 8 else 1   # AllGather chunks per layer
    Rc = R // AGC               # rows per AG chunk
    TPC = Rc // P               # l-tiles per AG chunk
    BST = min(512, E)           # bn_stats chunk
    NST = E // BST
    assert SAG == FTL and TPC % FTL == 0 and SC % P == 0

    # ---- DRAM I/O ----
    xown_f32 = nc.dram_tensor("xown_f32", [P, LT, E], F32, kind="ExternalInput").ap()
    xown_bf = nc.dram_tensor("xown_bf", [R, E], BF16, kind="ExternalInput").ap()
    xfull_bf = nc.dram_tensor("xfull_bf", [L, E], BF16, kind="ExternalInput").ap()
    wqR_in = nc.dram_tensor("wqR", [D, ET, P, ET * P], BF16, kind="ExternalInput").ap()
    wkR_in = nc.dram_tensor("wkR", [D, ET, P, ET * P], BF16, kind="ExternalInput").ap()
    w1T_in = nc.dram_tensor("w1T", [D, P, ET, F], BF16, kind="ExternalInput").ap()
    w2R_in = nc.dram_tensor("w2R", [D, ET, P, FT * P], BF16, kind="ExternalInput").ap()
    rel_in = nc.dram_tensor("relx", [LT, P, L], BF16, kind="ExternalInput").ap()
    out_own = nc.dram_tensor("out_own", [P, LT, E], F32, kind="ExternalOutput").ap()

    groups = [
        [c.n_pair * b + i for i in range(c.n_pair)] for b in range(c.B)
    ] if c.n_pair > 1 else None

    from contextlib import ExitStack

    with tile.TileContext(nc) as tc, ExitStack() as ctx:
        singles = ctx.enter_context(tc.tile_pool(name="singles", bufs=1))
        dram = ctx.enter_context(tc.tile_pool(name="dram", bufs=1, space="DRAM"))
        ps_sc = ctx.enter_context(tc.tile_pool(name="ps_sc", bufs=2, space="PSUM"))
        ps_sa = ctx.enter_context(tc.tile_pool(name="ps_sa", bufs=2, space="PSUM"))
        ps_mm = ctx.enter_context(tc.tile_pool(name="ps_mm", bufs=2, space="PSUM"))
        hTp = ctx.enter_context(tc.tile_pool(name="hTp", bufs=2))
        h2Tp = ctx.enter_context(tc.tile_pool(name="h2Tp", bufs=2))
        eplp = ctx.enter_context(tc.tile_pool(name="eplp", bufs=3))
        accp = ctx.enter_context(tc.tile_pool(name="accp", bufs=2))
        relp = ctx.enter_context(tc.tile_pool(name="relp", bufs=2))
        ztp = ctx.enter_context(tc.tile_pool(name="ztp", bufs=2))
        h2sp = ctx.enter_context(tc.tile_pool(name="h2sp", bufs=2))
        h2mp = ctx.enter_context(tc.tile_pool(name="h2mp", bufs=2))
        hmtp = ctx.enter_context(tc.tile_pool(name="hmtp", bufs=3))
        wqp = ctx.enter_context(tc.tile_pool(name="wqp", bufs=2))
        w1p = ctx.enter_context(tc.tile_pool(name="w1p", bufs=2))
        w2p = ctx.enter_context(tc.tile_pool(name="w2p", bufs=2))
        smalls = ctx.enter_context(tc.tile_pool(name="smalls", bufs=2))

        # ---- persistent slabs ----
        h_own = singles.tile([P, LT, E], F32, name="h_own")
        KT = singles.tile([P, ET, L], BF16, name="KT")
        QT = singles.tile([P, ET, R], BF16, name="QT")
        # slabW: [ff1T | ffX a/b | ff_row a/b]
        slabW = singles.tile([P, FT * Lh + 2 * ET * Lh + 2 * E], BF16, name="slabW")
        ff1T = slabW[:, : FT * Lh].rearrange("p (a b) -> p a b", a=FT)
        _o1 = FT * Lh
        ffX = [
            slabW[:, _o1 : _o1 + ET * Lh].rearrange("p (a b) -> p a b", a=ET),
            slabW[:, _o1 + ET * Lh : _o1 + 2 * ET * Lh].rearrange(
                "p (a b) -> p a b", a=ET
            ),
        ]
        _o2 = _o1 + 2 * ET * Lh
        ff_row = [slabW[:, _o2 : _o2 + E], slabW[:, _o2 + E : _o2 + 2 * E]]

        # persistent smalls
        mv_all = singles.tile([P, LT, 2], F32, name="mv_all")
        rstd_all = singles.tile([P, LT], F32, name="rstd_all")
        rs2 = singles.tile([P, LT], F32, name="rs2")
        recip2 = singles.tile([P, LT], F32, name="recip2")
        c_eps = singles.tile([P, 1], F32, name="c_eps")
        nc.vector.memset(c_eps, float(EPS))
        c_eps2 = singles.tile([P, 1], F32, name="c_eps2")
        nc.vector.memset(c_eps2, float(EPS * EPS))

        # Pre-load the one ACT table set containing Exp, Ln, Copy and Relu so
        # the act-table insertion pass never needs to switch sets mid-kernel.
        from concourse.hw_specs import get_activation_tables

        _tables = list(get_activation_tables(nc.m.arch))
        _set_id = _tables.index("natural_log_exp_and_others")
        nc.scalar.add_instruction(
            mybir.InstLoadActFuncSet(
                name=nc.get_next_instruction_name(),
                act_func_set_id=_set_id,
                ins=[],
                outs=[],
            )
        )

        # DRAM exchange buffers (double-buffered across layers)
        hout_d = [
            dram.tile([R, E], BF16, name="hout0"),
            dram.tile([R, E], BF16, name="hout1"),
        ]
        hf_ch = [
            [dram.tile([c.n_pair, Rc, E], BF16, name=f"hf{i}_{a}") for a in range(AGC)]
            for i in range(2)
        ]

        # init residual
        nc.sync.dma_start(out=h_own[:], in_=xown_f32[:])

        def msrc(d, mt, row0, nrows, col0, ncols):
            """AP for rows [mt*P+row0, +nrows), cols [col0,+ncols) of layer-d
            input stream (global row order)."""
            if d == 0:
                g0 = mt * P + row0
                return xfull_bf[g0 : g0 + nrows, col0 : col0 + ncols]
            half, within = divmod(mt, MT // c.n_pair)
            a = within // TPC
            loc = within * P - a * Rc + row0
            return hf_ch[(d - 1) % 2][a][
                half, loc : loc + nrows, col0 : col0 + ncols
            ]

        def emit_qproj(d_, ci):
            # Q^T columns for own-row chunk ci of layer d_ (reads hout of d_-1)
            qsrc = xown_bf if d_ == 0 else hout_d[(d_ - 1) % 2]
            hTc = hTp.tile([P, ET, SC], BF16, tag="hT", name="hTq")
            nc.sync.dma_start_transpose(hTc[:], qsrc[ci * SC : (ci + 1) * SC, :])
            for ept in range(ET):
                wqb = wqp.tile([P, ET, P], BF16, tag="wq", name="wqb")
                nc.sync.dma_start(
                    out=wqb[:],
                    in_=wqR_in[d_, ept].rearrange("p (a b) -> p a b", a=ET),
                )
                ps = ps_mm.tile([P, 512], F32, tag="mm", name="ps_q")
                for et in range(ET):
                    nc.tensor.matmul(
                        ps[:, :SC],
                        wqb[:, et, :],
                        hTc[:, et, :],
                        start=(et == 0),
                        stop=(et == ET - 1),
                    )
                nc.scalar.copy(
                    out=QT[:, ept, ci * SC : (ci + 1) * SC], in_=ps[:, :SC]
                )

        for d in range(D):
            last = d == D - 1

            if d == 0:
                for ci in range(NSCq):
                    emit_qproj(0, ci)

            # ---- K projection: KT[e', m] over full L (AG chunk order) ----
            ci_order = [a + g * AGC for a in range(AGC) for g in range(NSC // AGC)] \
                if d > 0 and AGC > 1 else list(range(NSC))
            # chunk ci covers global rows [ci*SC, (ci+1)*SC) = m-tiles
            # [ci*SC//P, ...). For d>0 those map into hf_ch[a] slices.
            for ci in ci_order:
                mt0 = ci * SC // P
                hTc = hTp.tile([P, ET, SC], BF16, tag="hT", name="hTk")
                nc.sync.dma_start_transpose(hTc[:], msrc(d, mt0, 0, SC, 0, E))
                for ept in range(ET):
                    wkb = wqp.tile([P, ET, P], BF16, tag="wq", name="wkb")
                    nc.sync.dma_start(
                        out=wkb[:],
                        in_=wkR_in[d, ept].rearrange("p (a b) -> p a b", a=ET),
                    )
                    ps = ps_mm.tile([P, 512], F32, tag="mm", name="ps_k")
                    for et in range(ET):
                        nc.tensor.matmul(
                            ps[:, :SC],
                            wkb[:, et, :],
                            hTc[:, et, :],
                            start=(et == 0),
                            stop=(et == ET - 1),
                        )
                    nc.scalar.copy(
                        out=KT[:, ept, ci * SC : (ci + 1) * SC], in_=ps[:, :SC]
                    )

            # ---- sweep over own l-tiles ----
            # PE is strict FIFO, so sa/FFN matmul work is queued as ~1-2us
            # slices and fed between the per-head score matmuls to keep the
            # scalar engine (softmax exp, the sweep bottleneck) streaming.
            from collections import deque

            pending = deque()

            def feed(n):
                for _ in range(n):
                    if pending:
                        pending.popleft()()

            zt_t = {}

            def make_sa_group(g_ts):
                """Queue sa matmuls + res1 + LN12 + h2T for a tile group."""
                h2Tc = h2Tp.tile([P, ET, Lh], BF16, tag="h2T", name="h2Tc")
                st = {"pss": {}}

                def open_eh(eh):
                    def f():
                        for tt in g_ts:
                            st["pss"][(eh, tt)] = ps_sa.tile(
                                [P, EH], F32, tag="sa", name="ps_sa"
                            )
                    return f

                def mm_slice(eh, mt0, nmt):
                    def f():
                        for mt in range(mt0, mt0 + nmt):
                            hmt = hmtp.tile([P, EH], BF16, tag="hmt", name="hmt")
                            nc.sync.dma_start(
                                out=hmt[:], in_=msrc(d, mt, 0, P, eh * EH, EH)
                            )
                            for tt in g_ts:
                                nc.tensor.matmul(
                                    st["pss"][(eh, tt)],
                                    zt_t[tt][:, mt, :],
                                    hmt[:],
                                    start=(mt == 0),
                                    stop=(mt == MT - 1),
                                )
                    return f

                def close_eh(eh):
                    def f():
                        for tt in g_ts:
                            nc.vector.scalar_tensor_tensor(
                                out=h_own[:, tt, eh * EH : (eh + 1) * EH],
                                in0=st["pss"][(eh, tt)],
                                scalar=recip2[:, tt : tt + 1],
                                in1=h_own[:, tt, eh * EH : (eh + 1) * EH],
                                op0=OP.mult,
                                op1=OP.add,
                            )
                    return f

                def ln12(tt):
                    def f():
                        stats = smalls.tile([P, NST, 6], F32, tag="st", name="st12")
                        for i in range(NST):
                            nc.vector.bn_stats(
                                out=stats[:, i, :],
                                in_=h_own[:, tt, i * BST : (i + 1) * BST],
                            )
                        nc.vector.bn_aggr(out=mv_all[:, tt, :], in_=stats[:])
                        lnv = smalls.tile([P, 1], F32, tag="lnv", name="lnv")
                        nc.scalar.activation(
                            out=lnv, in_=mv_all[:, tt, 1:2], func=AF.Ln,
                            bias=c_eps2, scale=float(1.0 + EPS),
                        )
                        nc.scalar.activation(
                            out=rstd_all[:, tt : tt + 1], in_=lnv, func=AF.Exp,
                            scale=-0.5,
                        )
                        h2st = h2sp.tile([P, E], BF16, tag="h2st", name="h2st")
                        nc.gpsimd.tensor_scalar(
                            out=h2st[:], in0=h_own[:, tt, :],
                            scalar1=mv_all[:, tt, 0:1],
                            scalar2=rstd_all[:, tt : tt + 1],
                            op0=OP.subtract, op1=OP.mult,
                        )
                        lloc = (tt % FTL) * P
                        nc.sync.dma_start_transpose(
                            h2Tc[:, :, lloc : lloc + P], h2st[:]
                        )
                    return f

                for eh in range(NEH):
                    pending.append(open_eh(eh))
                    for mt0 in range(0, MT, 4):
                        pending.append(mm_slice(eh, mt0, min(4, MT - mt0)))
                    pending.append(close_eh(eh))
                for tt in g_ts:
                    pending.append(ln12(tt))
                return h2Tc

            def make_ffn_chunk(fc, h2Tc):
                ch_ts = list(range(fc * FTL, (fc + 1) * FTL))
                st = {}

                def f1_slice(ft2):
                    # one weight DMA covers two f-tiles (contiguous in F)
                    def f():
                        w1b = w1p.tile([P, ET, 2 * P], BF16, tag="w1", name="w1b")
                        nc.sync.dma_start(
                            out=w1b[:],
                            in_=w1T_in[d, :, :, ft2 * 2 * P : (ft2 + 1) * 2 * P],
                        )
                        st["w1b"] = w1b

                    def g(ft):
                        def h():
                            w1b = st["w1b"]
                            ps = ps_mm.tile([P, 512], F32, tag="mm", name="ps_f1")
                            for et in range(ET):
                                nc.tensor.matmul(
                                    ps[:, :Lh],
                                    w1b[:, et, (ft % 2) * P : (ft % 2 + 1) * P],
                                    h2Tc[:, et, :],
                                    start=(et == 0),
                                    stop=(et == ET - 1),
                                )
                            nc.vector.tensor_scalar(
                                out=ff1T[:, ft, :], in0=ps[:, :Lh], scalar1=0.0,
                                scalar2=None, op0=OP.max,
                            )
                        return h
                    return f, g

                def f2_slice(ept, fh):
                    def f():
                        if fh == 0:
                            st[ept] = ps_mm.tile([P, 512], F32, tag="mm", name="ps_f2")
                            w2b = w2p.tile([P, FT, P], BF16, tag="w2", name="w2b")
                            nc.sync.dma_start(
                                out=w2b[:],
                                in_=w2R_in[d, ept].rearrange(
                                    "p (a b) -> p a b", a=FT
                                ),
                            )
                            st[("w", ept)] = w2b
                        ps2 = st[ept]
                        w2b = st[("w", ept)]
                        for f2 in range(FH2):
                            nc.tensor.matmul(
                                ps2[:, :Lh],
                                w2b[:, fh * FH2 + f2, :],
                                ff1T[:, fh * FH2 + f2, :],
                                start=(fh == 0 and f2 == 0),
                                stop=(fh == NFH - 1 and f2 == FH2 - 1),
                            )
                        if fh == NFH - 1:
                            nc.vector.tensor_copy(
                                out=ffX[fc % 2][:, ept, :], in_=ps2[:, :Lh]
                            )
                    return f

                def res2_ln3(tt):
                    def f():
                        fx = ffX[fc % 2]
                        fr = ff_row[tt % 2]
                        lloc = (tt % FTL) * P
                        for et in range(ET):
                            nc.sync.dma_start_transpose(
                                fr[:, et * P : (et + 1) * P],
                                fx[:, et, lloc : lloc + P],
                            )
                        h2mt = h2mp.tile([P, E], BF16, tag="h2mt", name="h2mt")
                        nc.gpsimd.tensor_scalar(
                            out=h2mt[:], in0=h_own[:, tt, :],
                            scalar1=mv_all[:, tt, 0:1],
                            scalar2=rstd_all[:, tt : tt + 1],
                            op0=OP.subtract, op1=OP.mult,
                        )
                        nc.vector.tensor_tensor(
                            out=h_own[:, tt, :], in0=fr[:], in1=h2mt[:], op=OP.add
                        )
                        stats = smalls.tile([P, NST, 6], F32, tag="st", name="st3")
                        for i in range(NST):
                            nc.vector.bn_stats(
                                out=stats[:, i, :],
                                in_=h_own[:, tt, i * BST : (i + 1) * BST],
                            )
                        mv3 = smalls.tile([P, 2], F32, tag="mv3", name="mv3")
                        nc.vector.bn_aggr(out=mv3[:], in_=stats[:])
                        lnv = smalls.tile([P, 1], F32, tag="lnv", name="lnv3")
                        nc.scalar.activation(
                            out=lnv, in_=mv3[:, 1:2], func=AF.Ln, bias=c_eps,
                            scale=1.0,
                        )
                        rstd3 = smalls.tile([P, 1], F32, tag="rstd3", name="rstd3")
                        nc.scalar.activation(
                            out=rstd3, in_=lnv, func=AF.Exp, scale=-0.5
                        )
                        nc.vector.tensor_scalar(
                            out=h_own[:, tt, :], in0=h_own[:, tt, :],
                            scalar1=mv3[:, 0:1], scalar2=rstd3,
                            op0=OP.subtract, op1=OP.mult,
                        )
                        if last:
                            nc.sync.dma_start(
                                out=out_own[:, tt, :], in_=h_own[:, tt, :]
                            )
                        else:
                            nc.gpsimd.dma_start(
                                out=hout_d[d % 2][tt * P : (tt + 1) * P, :],
                                in_=h_own[:, tt, :],
                            )
                    return f

                def qproj_kick(ci):
                    def f():
                        emit_qproj(d + 1, ci)
                    return f

                def ag_kick(a):
                    def f():
                        src = hout_d[d % 2][a * Rc : (a + 1) * Rc]
                        dst = hf_ch[d % 2][a]
                        if no_collective or groups is None:
                            for i in range(c.n_pair):
                                nc.sync.dma_start(out=dst[i], in_=src)
                        else:
                            nc.gpsimd.collective_compute(
                                "AllGather",
                                OP.bypass,
                                replica_groups=groups,
                                ins=[src.opt()],
                                outs=[dst.opt()],
                            )
                    return f

                for ft in range(FT):
                    if ft % 2 == 0:
                        ld, g = f1_slice(ft // 2)
                        pending.append(ld)
                        pending.append(g(ft))
                    else:
                        pending.append(g(ft))
                for ept in range(ET):
                    for fh in range(NFH):
                        pending.append(f2_slice(ept, fh))
                for tt in ch_ts:
                    pending.append(res2_ln3(tt))
                if not last and (ch_ts[-1] + 1) % TPC == 0:
                    pending.append(ag_kick((ch_ts[-1] + 1) // TPC - 1))
                if not last and ((ch_ts[-1] + 1) * P) % SC == 0:
                    pending.append(qproj_kick((ch_ts[-1] + 1) * P // SC - 1))

            for t in range(LT):
                rel = relp.tile([P, L], BF16, tag="rel", name="rel")
                nc.sync.dma_start(out=rel[:], in_=rel_in[t, :, :])
                rs_parts = smalls.tile([P, H, max(NCH, 1)], F32, tag="rsp", name="rsp")
                acc = accp.tile([P, L], BF16, tag="acc", name="acc")

                for h in range(H):
                    poff = c.DH * (h % HPT)
                    ept = h // HPT
                    qs = QT[poff : poff + c.DH, ept, t * P : (t + 1) * P]
                    e_pl = eplp.tile([P, L], BF16, tag="epl", name="e_pl")
                    for ch in range(NCH):
                        pssc = ps_sc.tile([P, SCCH], F32, tag="sc", name="pssc")
                        for j in range(NJ):
                            m0 = ch * SCCH + j * JW
                            nc.tensor.matmul(
                                pssc[:, j * JW : (j + 1) * JW],
                                qs,
                                KT[poff : poff + c.DH, ept, m0 : m0 + JW],
                                start=True,
                                stop=True,
                            )
                        nc.scalar.activation(
                            out=e_pl[:, ch * SCCH : (ch + 1) * SCCH],
                            in_=pssc[:],
                            func=AF.Exp,
                            scale=ISD,
                            accum_out=rs_parts[:, h, ch : ch + 1],
                        )
                    rsh = smalls.tile([P, 1], F32, tag="rsh", name="rsh")
                    if NCH > 1:
                        nc.vector.tensor_reduce(
                            out=rsh, in_=rs_parts[:, h, :],
                            axis=mybir.AxisListType.X, op=OP.add,
                        )
                    else:
                        nc.vector.tensor_copy(out=rsh, in_=rs_parts[:, h, :])
                    nc.vector.reciprocal(out=rsh, in_=rsh)
                    eng = nc.gpsimd if h in pool_set else nc.vector
                    if h == 0:
                        nc.vector.tensor_scalar(
                            out=acc[:], in0=e_pl[:], scalar1=rsh, scalar2=CH,
                            op0=OP.mult, op1=OP.mult,
                        )
                    else:
                        eng.tensor_scalar(
                            out=e_pl[:], in0=e_pl[:], scalar1=rsh, scalar2=CH,
                            op0=OP.mult, op1=OP.mult,
                        )
                        eng.tensor_tensor(
                            out=acc[:], in0=e_pl[:], in1=acc[:], op=OP.add
                        )
                    feed(2 if len(pending) <= 12 else 3)

                # stage 2: z = exp(acc + rel), in place in acc
                nc.vector.tensor_tensor(out=acc[:], in0=rel[:], in1=acc[:], op=OP.add)
                nc.scalar.activation(
                    out=acc[:], in_=acc[:], func=AF.Exp, scale=1.0,
                    accum_out=rs2[:, t : t + 1],
                )
                nc.vector.reciprocal(
                    out=recip2[:, t : t + 1], in_=rs2[:, t : t + 1]
                )
                zt = ztp.tile([P, MT, P], BF16, tag="zt", name="zt")
                zt_t[t] = zt
                nc.sync.dma_start_transpose(zt[:], acc[:])
                feed(2)

                if (t + 1) % SAG == 0:
                    h2Tc = make_sa_group(list(range(t + 1 - SAG, t + 1)))
                if (t + 1) % FTL == 0:
                    make_ffn_chunk((t + 1) // FTL - 1, h2Tc)

            # drain remaining sa/FFN/AG work
            while pending:
                pending.popleft()()


# ---------------- host-side helpers ----------------

def make_rel(L):
    pos = np.arange(L)
    return np.exp(-np.abs(pos[:, None] - pos[None, :]).astype(np.float32) / DECAY)


def ml_bf16():
    import ml_dtypes

    return ml_dtypes.bfloat16


def prep_inputs(cfg: Cfg, inputs):
    """inputs: dict of full numpy arrays as in reference.setup_inputs().
    Returns list of per-core in_maps."""
    c = cfg
    x = np.asarray(inputs["x"], np.float32)
    Wq = np.asarray(inputs["Wq"], np.float32)
    Wk = np.asarray(inputs["Wk"], np.float32)
    W1 = np.asarray(inputs["W1"], np.float32)
    W2 = np.asarray(inputs["W2"], np.float32)
    rel = make_rel(c.L)

    def to_lhsT(w):  # [D, out, in] -> [D, P, in_tiles, out] (w.T on partitions)
        D_, O_, I_ = w.shape
        wT = np.ascontiguousarray(np.transpose(w, (0, 2, 1)))  # [D, in, out]
        return wT.reshape(D_, I_ // P, P, O_).transpose(0, 2, 1, 3).astype(ml_bf16())

    def to_blkT(w):
        # [D, O, I] -> [D, O/P, P, (I/P)*P]; blk[d, ot, p, it*P+j] = w[d, ot*P+j, it*P+p]
        D_, O_, I_ = w.shape
        return np.ascontiguousarray(
            w.reshape(D_, O_ // P, P, I_ // P, P).transpose(0, 1, 4, 3, 2)
            .reshape(D_, O_ // P, P, I_)
        ).astype(ml_bf16())

    wqR = to_blkT(Wq)
    wkR = to_blkT(Wk)
    w1T = to_lhsT(W1)
    w2R = to_blkT(W2)

    in_maps = []
    for core in range(c.n_cores):
        b = core // c.n_pair
        s = core % c.n_pair
        R0 = s * c.R
        xrows = x[b, R0 : R0 + c.R]  # [R, E]
        xown_f32 = np.ascontiguousarray(
            xrows.reshape(c.LT, P, c.E).transpose(1, 0, 2)
        )
        relx = np.ascontiguousarray(
            rel[R0 : R0 + c.R].reshape(c.LT, P, c.L)
        ).astype(ml_bf16())
        in_maps.append(
            {
                "xown_f32": xown_f32,
                "xown_bf": xrows.astype(ml_bf16()),
                "xfull_bf": x[b].astype(ml_bf16()),
                "wqR": wqR,
                "wkR": wkR,
                "w1T": w1T,
                "w2R": w2R,
                "relx": relx,
            }
        )
    return in_maps


def assemble(cfg: Cfg, results):
    """results: list of per-core {'out_own': [P, LT, E]} -> full [B, L, E]."""
    c = cfg
    out = np.zeros((c.B, c.L, c.E), np.float32)
    for core in range(c.n_cores):
        b = core // c.n_pair
        s = core % c.n_pair
        R0 = s * c.R
        oo = results[core]["out_own"]
        out[b, R0 : R0 + c.R] = oo.transpose(1, 0, 2).reshape(c.R, c.E)
    return out


# ---------------- public entry ----------------

_CACHE = {}


def _get_nc(cfg: Cfg):
    key = ("nc", cfg.L, cfg.D, cfg.E)
    if key not in _CACHE:
        import concourse.bacc as bacc

        nc = bacc.Bacc(
            "TRN2", target_bir_lowering=False, debug=False, num_devices=cfg.n_cores
        )
        build_decoder(nc, cfg)
        nc.compile()
        _CACHE[key] = nc
    return _CACHE[key]


def run(inputs, cfg: Cfg = FULL, trace: bool = False, **spmd_kwargs):
    from concourse.bass_utils import run_bass_kernel_spmd

    nc = _get_nc(cfg)
    in_maps = prep_inputs(cfg, inputs)
    res = run_bass_kernel_spmd(
        nc, in_maps, core_ids=list(range(cfg.n_cores)), trace=trace, **spmd_kwargs
    )
    out = assemble(cfg, res.results)
    return out, res


def kernel(**inputs):
    out, _ = run(inputs)
    return out.astype(np.float32)
